# revision 1
# baseline (speedup 1.0000x reference)
"""BEiT self-attention Trainium2 kernel (Bass/Tile), data-parallel over batch on 8 cores.

v5 layout strategy (per core, 8 batches), all matmuls in bf16:
  - hidden pre-transposed on host to feature-major xT [768, 1576] bf16.
  - Q^T, K^T computed head-dim-major [o, m] into bf16 SBUF tiles (PSUM f32
    accumulate, cast on copy-out). 1/sqrt(64) folded into wq/bq on host.
    kt is padded +59 columns (zeros) so the jt=1 scores stationary can take a
    full 128-wide slice for every batch.
  - V computed seq-major per (batch, jt) with a ones column per head
    (65-wide head groups) so the probs@V matmul also yields softmax row-sums.
  - Attention per (batch, head): scoresT[j, i] for BOTH j-tiles of one head
    land in one PSUM bank (columns 0:208 / 208:416) — both matmuls share the
    PE row offset; mixing row offsets within a bank wedges the device.
    ACT applies exp() from PSUM -> f32; DVE (or GPSIMD for odd heads)
    multiplies by host-precomputed exp(rel_bias) bf16 -> u bf16.
    (All-bf16 DVE tensor_tensor mis-executes on HW; f32 src0 forces the
    working 1x path. ACT with a strided dst also mis-executes -> v copy on
    DVE.)
  - ctx computed directly seq-major: out[i, h-block] += u-slice.T @ v[j,:]
    accumulated over jt into two PSUM banks (i rows 0-127 / 128-196) per
    6-head group; after 6 heads: reciprocal of the ones-column sums + DVE
    scale -> bf16 ob, DMA out (host upcasts).
  - bv is NOT applied on device: sum(probs)=1 so out += bv is exact; the
    host adds bv after the gather.
  - A one-element exp at t~0 preloads the ACT exp table set so the ~2.7us
    table DMA doesn't stall the ACT queue mid-kernel.
  - Work split into 4 groups of 2 batches; group g's attention interleaves
    with group g+1's projection matmuls to keep the PE stream dense.
"""

from collections import deque

import numpy as np
import ml_dtypes

import concourse.bacc as bacc
import concourse.mybir as mybir
from concourse.tile import TileContext
from concourse.bass import broadcast_tensor_aps as bass_broadcast
from concourse.bass_utils import run_bass_kernel_spmd

B, S, D, H, HD = 64, 197, 768, 12, 64
NCORES = 8
BPC = B // NCORES  # batches per core
F32 = mybir.dt.float32
BF16 = mybir.dt.bfloat16
IW = 200  # padded query (i) window
PW = 2 * IW  # both j-tiles of one head side by side
KT = D // 128  # 6 contraction tiles
OT = D // 128  # 6 output-feature tiles
JT = [(0, 128), (128, S - 128)]
AluOp = mybir.AluOpType
ActFn = mybir.ActivationFunctionType
GPS_HEADS = (1, 3, 5, 7, 9, 11)  # heads whose u-multiply runs on GPSIMD


def build_program(bpc=BPC, group_sizes=None):
    if group_sizes is None:
        group_sizes = (2, 2, 2, 2) if bpc == 8 else (bpc,)
    assert sum(group_sizes) == bpc
    MTOT = bpc * S

    nc = bacc.Bacc("TRN2", target_bir_lowering=False, debug=False, num_devices=1)
    xT_d = nc.dram_tensor("xT", [D, MTOT], BF16, kind="ExternalInput")
    wqT_d = nc.dram_tensor("wqT", [D, D], BF16, kind="ExternalInput")
    wkT_d = nc.dram_tensor("wkT", [D, D], BF16, kind="ExternalInput")
    wvT_d = nc.dram_tensor("wvT", [D, D], BF16, kind="ExternalInput")
    bq_d = nc.dram_tensor("bq2", [128, OT], F32, kind="ExternalInput")
    eb_d = nc.dram_tensor("ebias", [H, 128, PW], BF16, kind="ExternalInput")
    out_d = nc.dram_tensor("out", [MTOT, D], BF16, kind="ExternalOutput")

    with TileContext(nc) as tc:
        with (
            tc.tile_pool(name="const", bufs=1) as cp,
            tc.tile_pool(name="grp", bufs=2) as gp,
            tc.tile_pool(name="work", bufs=3) as wp,
            tc.tile_pool(name="ps", bufs=1, space="PSUM") as pp,
        ):
            wq_t = [
                cp.tile([128, D], BF16, name=f"wq{k}", tag=f"wq{k}") for k in range(KT)
            ]
            wk_t = [
                cp.tile([128, D], BF16, name=f"wk{k}", tag=f"wk{k}") for k in range(KT)
            ]
            wv_t = [
                cp.tile([128, D], BF16, name=f"wv{k}", tag=f"wv{k}") for k in range(KT)
            ]
            bqs = cp.tile([128, OT], F32, tag="bqs")
            warm = cp.tile([128, 8], F32, tag="warm")
            eb_t = {}
            for h in range(H):
                eb_t[h] = cp.tile([128, PW], BF16, name=f"eb{h}", tag=f"eb{h}")

            # ACT exp-table preload at t~0
            nc.vector.memset(warm[:, :], 0.0)
            nc.scalar.activation(warm[:, 0:1], warm[:, 4:5], ActFn.Exp)

            # PE warm-up: keep the PE array busy (and the clock ramped) while
            # the first weight/x DMAs are in flight. Results are discarded.
            wmm = cp.tile([128, 512], BF16, tag="wmm")
            nc.vector.memset(wmm[:, :], 0.0)
            for _ in range(24):
                wps = pp.tile([128, 512], F32, name="pp", tag="mm512", bufs=4)
                nc.tensor.matmul(
                    wps[:, 0:512], wmm[:, 0:128], wmm[:, 0:512], start=True, stop=True
                )

            def load_wq():
                nc.sync.dma_start(bqs[:], bq_d[:, :])
                for k in range(KT):
                    nc.sync.dma_start(wq_t[k][:], wqT_d[k * 128 : (k + 1) * 128, :])

            def load_weights():
                for k in range(KT):
                    nc.sync.dma_start(wk_t[k][:], wkT_d[k * 128 : (k + 1) * 128, :])
                for k in range(KT):
                    nc.sync.dma_start(wv_t[k][:], wvT_d[k * 128 : (k + 1) * 128, :])
                    nc.sync.dma_start(eb_t[2 * k][:, :], eb_d[2 * k, :, :])
                    nc.sync.dma_start(eb_t[2 * k + 1][:, :], eb_d[2 * k + 1, :, :])

            def proj_pieces(g, GB, b0):
                """Emission thunks for group g's projections; last item is the
                ('ctx', dict) sentinel carrying the produced tiles."""
                MG = GB * S
                QW = MG + (IW - S)
                KW = MG + (256 - S)
                m0 = b0 * S
                ctx = {}

                def piece_load():
                    xt = [
                        gp.tile([128, MG], BF16, name=f"xt{k}", tag=f"xt{k}")
                        for k in range(KT)
                    ]
                    for k in range(KT):
                        nc.sync.dma_start(
                            xt[k][:], xT_d[k * 128 : (k + 1) * 128, m0 : m0 + MG]
                        )
                    ctx["xt"] = xt
                    ctx["qt"] = [
                        gp.tile([128, QW], BF16, name=f"qt{o}", tag=f"qt{o}")
                        for o in range(OT)
                    ]
                    ctx["kt"] = [
                        gp.tile([128, KW], BF16, name=f"kt{o}", tag=f"kt{o}")
                        for o in range(OT)
                    ]
                    ctx["vt"] = {}

                yield piece_load

                def piece_qt(o):
                    xt, qt = ctx["xt"], ctx["qt"]
                    ps = pp.tile([128, 512], F32, name="pp", tag="mm512", bufs=4)
                    for ki in range(KT):
                        nc.tensor.matmul(
                            ps[:, :MG],
                            wq_t[ki][:, o * 128 : (o + 1) * 128],
                            xt[ki][:, :],
                            start=(ki == 0),
                            stop=(ki == KT - 1),
                        )
                    nc.scalar.activation(
                        qt[o][:, :MG],
                        ps[:, :MG],
                        ActFn.Identity,
                        bias=bqs[:, o : o + 1],
                    )
                    nc.vector.memset(qt[o][:, MG:QW], 0.0)

                def piece_kt(o):
                    xt, kt = ctx["xt"], ctx["kt"]
                    ps = pp.tile([128, 512], F32, name="pp", tag="mm512", bufs=4)
                    for ki in range(KT):
                        nc.tensor.matmul(
                            ps[:, :MG],
                            wk_t[ki][:, o * 128 : (o + 1) * 128],
                            xt[ki][:, :],
                            start=(ki == 0),
                            stop=(ki == KT - 1),
                        )
                    nc.vector.tensor_copy(kt[o][:, :MG], ps[:, :MG])
                    nc.vector.memset(kt[o][:, MG:KW], 0.0)

                def piece_v(b, jt):
                    xt = ctx["xt"]
                    j0, jw = JT[jt]
                    v = gp.tile(
                        [128, H * 65], BF16, name=f"v{b}_{jt}", tag=f"v{b}_{jt}"
                    )
                    v3 = v[:jw, :].rearrange("p (h c) -> p h c", c=65)
                    for c in range(2):
                        ps = pp.tile([128, 512], F32, name="pp", tag="mm512", bufs=4)
                        for ki in range(KT):
                            nc.tensor.matmul(
                                ps[:jw, :384],
                                xt[ki][:, b * S + j0 : b * S + j0 + jw],
                                wv_t[ki][:, c * 384 : (c + 1) * 384],
                                start=(ki == 0),
                                stop=(ki == KT - 1),
                            )
                        nc.vector.tensor_copy(
                            v3[:, c * 6 : (c + 1) * 6, 0:64],
                            ps[:jw, :384].rearrange("p (h c) -> p h c", c=64),
                        )
                    nc.vector.memset(v3[:, :, 64:65], 1.0)
                    ctx["vt"][b, jt] = v

                for o in range(OT):
                    yield (lambda o=o: piece_qt(o))
                for o in range(OT):
                    yield (lambda o=o: piece_kt(o))
                for b in range(GB):
                    for jt in range(2):
                        yield (lambda b=b, jt=jt: piece_v(b, jt))
                yield ("ctx", ctx)

            def att_pieces(GB, b0, ctx):
                """Emission thunks for a group's attention, pipelined by
                head-pair. ctx is read lazily (tiles created mid-stream)."""
                hgstate = {}

                def stage_pair(b, hp):
                    qt, kt = ctx["qt"], ctx["kt"]
                    o = hp // 2
                    us = []
                    for dh in (0, 1):
                        h = hp + dh
                        po = dh * 64
                        ps = pp.tile([128, 512], F32, name="st", tag="mm512", bufs=4)
                        for jt in (0, 1):
                            nc.tensor.matmul(
                                ps[:128, jt * IW : (jt + 1) * IW],
                                kt[o][po : po + 64, b * S + jt * 128 : b * S + jt * 128 + 128],
                                qt[o][po : po + 64, b * S : b * S + IW],
                                start=True,
                                stop=True,
                            )
                        e = wp.tile([128, PW], F32, name="e", tag="e", bufs=5)
                        nc.scalar.activation(e[:, :], ps[:128, :PW], ActFn.Exp)
                        u = wp.tile([128, PW], BF16, name=f"u{dh}", tag=f"u{dh}", bufs=8)
                        eng = nc.gpsimd if h in GPS_HEADS else nc.vector
                        eng.tensor_tensor(u[:, :], e[:, :], eb_t[h][:, :], AluOp.mult)
                        us.append(u)
                    return us

                def stage_ctx(b, hp, us, row0):
                    vt = ctx["vt"]
                    for dh in (0, 1):
                        h = hp + dh
                        hg, hl = h // 6, h % 6
                        if hl == 0:
                            ptA = pp.tile([128, 390], F32, name="ptA", tag="ptA", bufs=2)
                            ptB = pp.tile([128, 390], F32, name="ptB", tag="ptB", bufs=2)
                            hgstate[b, hg] = (ptA, ptB)
                        ptA, ptB = hgstate[b, hg]
                        for pt_t, i0, iw2 in [(ptA, 0, 128), (ptB, 128, S - 128)]:
                            for jt, (j0, jw) in enumerate(JT):
                                nc.tensor.matmul(
                                    pt_t[:iw2, hl * 65 : (hl + 1) * 65],
                                    us[dh][:jw, jt * IW + i0 : jt * IW + i0 + iw2],
                                    vt[b, jt][:jw, h * 65 : (h + 1) * 65],
                                    start=(jt == 0),
                                    stop=(jt == 1),
                                )
                        if hl == 5:
                            for pt_t, i0, iw2 in [(ptA, 0, 128), (ptB, 128, S - 128)]:
                                g3 = pt_t[:iw2, :].rearrange("p (h c) -> p h c", c=65)
                                rt = wp.tile([128, 6], F32, name="rt", tag="rt")
                                rt3 = rt[:iw2, :].rearrange("p (h c) -> p h c", c=1)
                                nc.vector.reciprocal(rt3, g3[:, :, 64:65])
                                num = g3[:, :, 0:64]
                                _, rb3 = bass_broadcast(num, rt3)
                                ob = wp.tile([128, 384], BF16, name="ob", tag="ob", bufs=6)
                                nc.vector.tensor_tensor(
                                    ob[:iw2, :].rearrange("p (h c) -> p h c", c=64),
                                    num,
                                    rb3,
                                    AluOp.mult,
                                )
                                nc.sync.dma_start(
                                    out_d[
                                        row0 + i0 : row0 + i0 + iw2,
                                        hg * 384 : (hg + 1) * 384,
                                    ],
                                    ob[:iw2, :],
                                )

                pend = deque()
                for b in range(GB):
                    for hp in range(0, H, 2):

                        def piece(b=b, hp=hp):
                            us = stage_pair(b, hp)
                            pend.append((b, hp, us, (b0 + b) * S))
                            while len(pend) > 2:
                                stage_ctx(*pend.popleft())

                        yield piece

                def flush():
                    while pend:
                        stage_ctx(*pend.popleft())

                yield flush

            def run_proj(gen):
                pieces = []
                ctx = None
                for item in gen:
                    if isinstance(item, tuple) and item[0] == "ctx":
                        ctx = item[1]
                    else:
                        pieces.append(item)
                return pieces, ctx

            b0s = []
            acc = 0
            for GB in group_sizes:
                b0s.append(acc)
                acc += GB

            g0_pieces, prev_ctx = run_proj(proj_pieces(0, group_sizes[0], b0s[0]))
            load_wq()
            g0_pieces[0]()  # xT DMAs right behind the wq tiles
            load_weights()
            for p in g0_pieces[1:]:
                p()

            def interleave(astream, pstream):
                ratio = max(1, len(astream) // max(1, len(pstream)))
                out = []
                ai = pi = 0
                while ai < len(astream) or pi < len(pstream):
                    for _ in range(ratio):
                        if ai < len(astream):
                            out.append(astream[ai])
                            ai += 1
                    if pi < len(pstream):
                        out.append(pstream[pi])
                        pi += 1
                return out

            ng = len(group_sizes)
            for g in range(1, ng - 1):
                pieces, g_ctx = run_proj(proj_pieces(g, group_sizes[g], b0s[g]))
                for p in interleave(
                    list(att_pieces(group_sizes[g - 1], b0s[g - 1], prev_ctx)), pieces
                ):
                    p()
                prev_ctx = g_ctx

            if ng == 1:
                for p in att_pieces(group_sizes[0], b0s[0], prev_ctx):
                    p()
            else:
                # final window: att(gl-1) interleaved with the last group's
                # load/qt/kt0-2 pieces; kt3-5 + V pieces are deferred into the
                # last group's own attention stream as just-in-time PE filler.
                gl = ng - 1
                pieces, gl_ctx = run_proj(proj_pieces(gl, group_sizes[gl], b0s[gl]))
                pload = pieces[0]
                pqt = pieces[1 : 1 + OT]
                pkt = pieces[1 + OT : 1 + 2 * OT]
                pv = deque(pieces[1 + 2 * OT :])
                window = [pload] + pqt + pkt[:3]
                for p in interleave(
                    list(att_pieces(group_sizes[gl - 1], b0s[gl - 1], prev_ctx)),
                    window,
                ):
                    p()
                apieces = list(att_pieces(group_sizes[gl], b0s[gl], gl_ctx))
                aflush = apieces[-1]
                A = apieces[:-1]
                out_stream = []
                for idx, a in enumerate(A):
                    if 3 <= idx < OT:
                        out_stream.append(pkt[idx])  # kt[idx] just before its pair
                    out_stream.append(a)
                    if pv and idx in (0, 1, 5, 7):
                        out_stream.append(pv.popleft())
                while pv:
                    out_stream.append(pv.popleft())
                out_stream.append(aflush)
                for p in out_stream:
                    p()

    nc.compile()
    return nc


def prep_host_inputs(inputs, bpc=BPC, cores=NCORES):
    """Shared (per-core-identical) tensors + per-core xT shards."""
    hs = np.ascontiguousarray(np.asarray(inputs["hidden_states"], dtype=np.float32))
    wq = np.asarray(inputs["wq"], np.float32)
    bq = np.asarray(inputs["bq"], np.float32)
    wk = np.asarray(inputs["wk"], np.float32)
    wv = np.asarray(inputs["wv"], np.float32)
    bias_table = np.asarray(inputs["bias_table"], np.float32)
    rel_index = np.asarray(inputs["rel_index"])

    bf = ml_dtypes.bfloat16
    scale = np.float32(1.0 / np.sqrt(HD))
    common = {
        "wqT": np.ascontiguousarray((wq.T * scale).astype(bf)),
        "wkT": np.ascontiguousarray(wk.T.astype(bf)),
        "wvT": np.ascontiguousarray(wv.T.astype(bf)),
        "bq2": np.ascontiguousarray((bq * scale).reshape(OT, 128).T),
    }
    rb = bias_table[rel_index]  # [i, j, H]
    ebT = np.exp(rb.transpose(2, 1, 0))  # [h, j, i]
    ebt = np.ones((H, 128, PW), np.float32)
    ebt[:, 0:128, 0:S] = ebT[:, 0:128, :]
    ebt[:, 0 : S - 128, IW : IW + S] = ebT[:, 128:S, :]
    common["ebias"] = np.ascontiguousarray(ebt.astype(bf))

    in_maps = []
    for c in range(cores):
        xc = hs[c * bpc : (c + 1) * bpc].reshape(bpc * S, D)
        in_maps.append({"xT": np.ascontiguousarray(xc.T.astype(bf)), **common})
    return in_maps


_prog_cache = {}


def get_program(bpc=BPC, group_sizes=None):
    key = (bpc, group_sizes)
    if key not in _prog_cache:
        _prog_cache[key] = build_program(bpc, group_sizes)
    return _prog_cache[key]


def kernel(**inputs):
    nc = get_program()
    in_maps = prep_host_inputs(inputs)
    res = run_bass_kernel_spmd(nc, in_maps, list(range(NCORES)))
    out = np.concatenate(
        [res.results[c]["out"].astype(np.float32) for c in range(NCORES)], axis=0
    )
    bv = np.asarray(inputs["bv"], np.float32)
    return out.reshape(B, S, D) + bv



# revision 2
# speedup vs baseline: 1.0068x; 1.0068x over previous
"""BEiT self-attention Trainium2 kernel (Bass/Tile), data-parallel over batch on 8 cores.

v6 layout strategy (per core, 8 batches), all matmuls in bf16:
  - hidden pre-transposed AND pre-packed on host: xP [128, KT*MG per group,
    groups side by side] bf16 so each group's x loads with ONE DMA
    (4.7KB rows). Weights packed the same way: wqw/wkw/wvw [128, KT*768]
    (one DMA each), ebias [128, H*PW] (one DMA). 9 input DMAs total --
    the Sync engine issues DMA descriptors serially at ~650ns each, so
    fewer/bigger DMAs move the first real matmul ~5us earlier.
  - Q^T, K^T computed head-dim-major [o, m] into bf16 SBUF tiles (PSUM f32
    accumulate, cast on copy-out). 1/sqrt(64) folded into wq/bq on host.
    kt is padded +59 columns (zeros) so the jt=1 scores stationary can take a
    full 128-wide slice for every batch.
  - V computed seq-major per (batch, jt) with a ones column per head
    (65-wide head groups) so the probs@V matmul also yields softmax row-sums.
  - Attention per (batch, head): scoresT[j, i] for BOTH j-tiles of one head
    land in one PSUM bank (columns 0:IW / IW:2IW, IW=197) -- both matmuls
    share the PE row offset; mixing row offsets within a bank wedges the
    device. ACT applies exp() from PSUM -> f32; DVE (or GPSIMD for odd
    heads) multiplies by host-precomputed exp(rel_bias) bf16 -> u bf16.
    (All-bf16 DVE tensor_tensor mis-executes on HW; f32 src0 forces the
    working 1x path. ACT with a strided dst also mis-executes -> v copy on
    DVE.)
  - ctx computed directly seq-major: out[i, h-block] += u-slice.T @ v[j,:]
    accumulated over jt into two PSUM banks (i rows 0-127 / 128-196) per
    6-head group; after 6 heads: reciprocal of the ones-column sums + DVE
    scale -> bf16 ob, DMA out (host upcasts).
  - bv is NOT applied on device: sum(probs)=1 so out += bv is exact; the
    host adds bv after the gather.
  - A one-element exp at t~0 preloads the ACT exp table set so the ~2.7us
    table DMA doesn't stall the ACT queue mid-kernel.
  - Work split into groups; group g's attention interleaves with group
    g+1's projection matmuls to keep the PE stream dense.
"""

from collections import deque

import numpy as np
import ml_dtypes

import concourse.bacc as bacc
import concourse.mybir as mybir
from concourse.tile import TileContext
from concourse.bass import broadcast_tensor_aps as bass_broadcast
from concourse.bass_utils import run_bass_kernel_spmd

B, S, D, H, HD = 64, 197, 768, 12, 64
NCORES = 8
BPC = B // NCORES  # batches per core
F32 = mybir.dt.float32
BF16 = mybir.dt.bfloat16
IW = 197  # query (i) window
PW = 2 * IW  # both j-tiles of one head side by side
KT = D // 128  # 6 contraction tiles
OT = D // 128  # 6 output-feature tiles
JT = [(0, 128), (128, S - 128)]
AluOp = mybir.AluOpType
ActFn = mybir.ActivationFunctionType
GPS_HEADS = (1, 3, 5, 7, 9, 11)  # heads whose u-multiply runs on GPSIMD
GROUP_SIZES = (2, 2, 2, 2)
WARMUP = 12


def build_program(bpc=BPC, group_sizes=GROUP_SIZES):
    assert sum(group_sizes) == bpc
    MTOT = bpc * S
    XPW = KT * MTOT  # packed x width

    nc = bacc.Bacc("TRN2", target_bir_lowering=False, debug=False, num_devices=1)
    xP_d = nc.dram_tensor("xP", [128, XPW], BF16, kind="ExternalInput")
    wqw_d = nc.dram_tensor("wqw", [128, KT * D], BF16, kind="ExternalInput")
    wkw_d = nc.dram_tensor("wkw", [128, KT * D], BF16, kind="ExternalInput")
    wvw_d = nc.dram_tensor("wvw", [128, KT * D], BF16, kind="ExternalInput")
    bq_d = nc.dram_tensor("bq2", [128, OT], F32, kind="ExternalInput")
    eb_d = nc.dram_tensor("ebias", [128, H * PW], BF16, kind="ExternalInput")
    out_d = nc.dram_tensor("out", [MTOT, D], BF16, kind="ExternalOutput")

    # group -> start offset (in packed x columns)
    b0s = []
    xoffs = []
    acc = 0
    xo = 0
    for GB in group_sizes:
        b0s.append(acc)
        xoffs.append(xo)
        acc += GB
        xo += KT * GB * S

    with TileContext(nc) as tc:
        with (
            tc.tile_pool(name="const", bufs=1) as cp,
            tc.tile_pool(name="grp", bufs=2) as gp,
            tc.tile_pool(name="work", bufs=3) as wp,
            tc.tile_pool(name="ps", bufs=1, space="PSUM") as pp,
        ):
            wqw = cp.tile([128, KT * D], BF16, tag="wqw")
            wkw = cp.tile([128, KT * D], BF16, tag="wkw")
            wvw = cp.tile([128, KT * D], BF16, tag="wvw")
            bqs = cp.tile([128, OT], F32, tag="bqs")
            warm = cp.tile([128, 8], F32, tag="warm")
            ebw = cp.tile([128, H * PW], BF16, tag="ebw")

            def wq_t(k):
                return wqw[:, k * D : (k + 1) * D]

            def wk_t(k):
                return wkw[:, k * D : (k + 1) * D]

            def wv_t(k):
                return wvw[:, k * D : (k + 1) * D]

            def eb_t(h):
                return ebw[:, h * PW : (h + 1) * PW]

            # ACT exp-table preload at t~0
            nc.vector.memset(warm[:, :], 0.0)
            nc.scalar.activation(warm[:, 0:1], warm[:, 4:5], ActFn.Exp)

            # PE warm-up: keep the PE array busy (and the clock ramped) while
            # the first weight/x DMAs are in flight. Results are discarded.
            wmm = cp.tile([128, 512], BF16, tag="wmm")
            nc.vector.memset(wmm[:, :], 0.0)
            for _ in range(WARMUP):
                wps = pp.tile([128, 512], F32, name="pp", tag="mm512", bufs=4)
                nc.tensor.matmul(
                    wps[:, 0:512], wmm[:, 0:128], wmm[:, 0:512], start=True, stop=True
                )

            def load_wq():
                nc.sync.dma_start(bqs[:], bq_d[:, :])
                nc.sync.dma_start(wqw[:], wqw_d[:, :])

            def load_weights():
                nc.sync.dma_start(wkw[:], wkw_d[:, :])
                nc.sync.dma_start(wvw[:], wvw_d[:, :])
                nc.sync.dma_start(ebw[:], eb_d[:, :])

            def proj_pieces(g, GB, b0):
                """Emission thunks for group g's projections; last item is the
                ('ctx', dict) sentinel carrying the produced tiles."""
                MG = GB * S
                KW = MG + (256 - S)
                ctx = {}

                def piece_load():
                    xt = gp.tile([128, KT * MG], BF16, name=f"xt{g}", tag="xt")
                    nc.sync.dma_start(
                        xt[:], xP_d[:, xoffs[g] : xoffs[g] + KT * MG]
                    )
                    ctx["xt"] = [xt[:, k * MG : (k + 1) * MG] for k in range(KT)]
                    ctx["qt"] = [
                        gp.tile([128, MG], BF16, name=f"qt{o}", tag=f"qt{o}")
                        for o in range(OT)
                    ]
                    ctx["kt"] = [
                        gp.tile([128, KW], BF16, name=f"kt{o}", tag=f"kt{o}")
                        for o in range(OT)
                    ]
                    ctx["vt"] = {}

                yield piece_load

                def piece_qt(o):
                    xt, qt = ctx["xt"], ctx["qt"]
                    ps = pp.tile([128, 512], F32, name="pp", tag="mm512", bufs=4)
                    for ki in range(KT):
                        nc.tensor.matmul(
                            ps[:, :MG],
                            wq_t(ki)[:, o * 128 : (o + 1) * 128],
                            xt[ki][:, :],
                            start=(ki == 0),
                            stop=(ki == KT - 1),
                        )
                    nc.scalar.activation(
                        qt[o][:, :MG],
                        ps[:, :MG],
                        ActFn.Identity,
                        bias=bqs[:, o : o + 1],
                    )

                def piece_kt(o):
                    xt, kt = ctx["xt"], ctx["kt"]
                    ps = pp.tile([128, 512], F32, name="pp", tag="mm512", bufs=4)
                    for ki in range(KT):
                        nc.tensor.matmul(
                            ps[:, :MG],
                            wk_t(ki)[:, o * 128 : (o + 1) * 128],
                            xt[ki][:, :],
                            start=(ki == 0),
                            stop=(ki == KT - 1),
                        )
                    nc.vector.tensor_copy(kt[o][:, :MG], ps[:, :MG])
                    nc.vector.memset(kt[o][:, MG:KW], 0.0)

                def piece_v(b, jt):
                    xt = ctx["xt"]
                    j0, jw = JT[jt]
                    v = gp.tile(
                        [128, H * 65], BF16, name=f"v{b}_{jt}", tag=f"v{b}_{jt}"
                    )
                    v3 = v[:jw, :].rearrange("p (h c) -> p h c", c=65)
                    for c in range(2):
                        ps = pp.tile([128, 512], F32, name="pp", tag="mm512", bufs=4)
                        for ki in range(KT):
                            nc.tensor.matmul(
                                ps[:jw, :384],
                                xt[ki][:, b * S + j0 : b * S + j0 + jw],
                                wv_t(ki)[:, c * 384 : (c + 1) * 384],
                                start=(ki == 0),
                                stop=(ki == KT - 1),
                            )
                        nc.vector.tensor_copy(
                            v3[:, c * 6 : (c + 1) * 6, 0:64],
                            ps[:jw, :384].rearrange("p (h c) -> p h c", c=64),
                        )
                    nc.vector.memset(v3[:, :, 64:65], 1.0)
                    ctx["vt"][b, jt] = v

                for o in range(OT):
                    yield (lambda o=o: piece_qt(o))
                for o in range(OT):
                    yield (lambda o=o: piece_kt(o))
                for b in range(GB):
                    for jt in range(2):
                        yield (lambda b=b, jt=jt: piece_v(b, jt))
                yield ("ctx", ctx)

            def att_pieces(GB, b0, ctx):
                """Emission thunks for a group's attention, pipelined by
                head-pair. ctx is read lazily (tiles created mid-stream)."""
                hgstate = {}

                def stage_pair(b, hp):
                    qt, kt = ctx["qt"], ctx["kt"]
                    o = hp // 2
                    us = []
                    for dh in (0, 1):
                        h = hp + dh
                        po = dh * 64
                        ps = pp.tile([128, 512], F32, name="st", tag="mm512", bufs=4)
                        for jt in (0, 1):
                            nc.tensor.matmul(
                                ps[:128, jt * IW : (jt + 1) * IW],
                                kt[o][po : po + 64, b * S + jt * 128 : b * S + jt * 128 + 128],
                                qt[o][po : po + 64, b * S : b * S + IW],
                                start=True,
                                stop=True,
                            )
                        e = wp.tile([128, PW], F32, name="e", tag="e", bufs=5)
                        nc.scalar.activation(e[:, :], ps[:128, :PW], ActFn.Exp)
                        u = wp.tile([128, PW], BF16, name=f"u{dh}", tag=f"u{dh}", bufs=8)
                        eng = nc.gpsimd if h in GPS_HEADS else nc.vector
                        eng.tensor_tensor(u[:, :], e[:, :], eb_t(h)[:, :], AluOp.mult)
                        us.append(u)
                    return us

                def stage_ctx(b, hp, us, row0):
                    vt = ctx["vt"]
                    for dh in (0, 1):
                        h = hp + dh
                        hg, hl = h // 6, h % 6
                        if hl == 0:
                            ptA = pp.tile([128, 390], F32, name="ptA", tag="ptA", bufs=2)
                            ptB = pp.tile([128, 390], F32, name="ptB", tag="ptB", bufs=2)
                            hgstate[b, hg] = (ptA, ptB)
                        ptA, ptB = hgstate[b, hg]
                        for pt_t, i0, iw2 in [(ptA, 0, 128), (ptB, 128, S - 128)]:
                            for jt, (j0, jw) in enumerate(JT):
                                nc.tensor.matmul(
                                    pt_t[:iw2, hl * 65 : (hl + 1) * 65],
                                    us[dh][:jw, jt * IW + i0 : jt * IW + i0 + iw2],
                                    vt[b, jt][:jw, h * 65 : (h + 1) * 65],
                                    start=(jt == 0),
                                    stop=(jt == 1),
                                )
                        if hl == 5:
                            for pt_t, i0, iw2 in [(ptA, 0, 128), (ptB, 128, S - 128)]:
                                g3 = pt_t[:iw2, :].rearrange("p (h c) -> p h c", c=65)
                                rt = wp.tile([128, 6], F32, name="rt", tag="rt")
                                rt3 = rt[:iw2, :].rearrange("p (h c) -> p h c", c=1)
                                nc.vector.reciprocal(rt3, g3[:, :, 64:65])
                                num = g3[:, :, 0:64]
                                _, rb3 = bass_broadcast(num, rt3)
                                ob = wp.tile([128, 384], BF16, name="ob", tag="ob", bufs=6)
                                nc.vector.tensor_tensor(
                                    ob[:iw2, :].rearrange("p (h c) -> p h c", c=64),
                                    num,
                                    rb3,
                                    AluOp.mult,
                                )
                                nc.sync.dma_start(
                                    out_d[
                                        row0 + i0 : row0 + i0 + iw2,
                                        hg * 384 : (hg + 1) * 384,
                                    ],
                                    ob[:iw2, :],
                                )

                pend = deque()
                for b in range(GB):
                    for hp in range(0, H, 2):

                        def piece(b=b, hp=hp):
                            us = stage_pair(b, hp)
                            pend.append((b, hp, us, (b0 + b) * S))
                            while len(pend) > 2:
                                stage_ctx(*pend.popleft())

                        yield piece

                def flush():
                    while pend:
                        stage_ctx(*pend.popleft())

                yield flush

            def run_proj(gen):
                pieces = []
                ctx = None
                for item in gen:
                    if isinstance(item, tuple) and item[0] == "ctx":
                        ctx = item[1]
                    else:
                        pieces.append(item)
                return pieces, ctx

            g0_pieces, prev_ctx = run_proj(proj_pieces(0, group_sizes[0], b0s[0]))
            load_wq()
            g0_pieces[0]()  # xP DMA right behind the wq tiles
            load_weights()
            for p in g0_pieces[1:]:
                p()

            def interleave(astream, pstream):
                ratio = max(1, len(astream) // max(1, len(pstream)))
                out = []
                ai = pi = 0
                while ai < len(astream) or pi < len(pstream):
                    for _ in range(ratio):
                        if ai < len(astream):
                            out.append(astream[ai])
                            ai += 1
                    if pi < len(pstream):
                        out.append(pstream[pi])
                        pi += 1
                return out

            ng = len(group_sizes)
            for g in range(1, ng - 1):
                pieces, g_ctx = run_proj(proj_pieces(g, group_sizes[g], b0s[g]))
                for p in interleave(
                    list(att_pieces(group_sizes[g - 1], b0s[g - 1], prev_ctx)), pieces
                ):
                    p()
                prev_ctx = g_ctx

            if ng == 1:
                for p in att_pieces(group_sizes[0], b0s[0], prev_ctx):
                    p()
            else:
                # final window: att(gl-1) interleaved with the last group's
                # load/qt/kt0-2 pieces; kt3-5 + V pieces are deferred into the
                # last group's own attention stream as just-in-time PE filler.
                gl = ng - 1
                pieces, gl_ctx = run_proj(proj_pieces(gl, group_sizes[gl], b0s[gl]))
                pload = pieces[0]
                pqt = pieces[1 : 1 + OT]
                pkt = pieces[1 + OT : 1 + 2 * OT]
                pv = deque(pieces[1 + 2 * OT :])
                window = [pload] + pqt + pkt[:3]
                for p in interleave(
                    list(att_pieces(group_sizes[gl - 1], b0s[gl - 1], prev_ctx)),
                    window,
                ):
                    p()
                apieces = list(att_pieces(group_sizes[gl], b0s[gl], gl_ctx))
                aflush = apieces[-1]
                A = apieces[:-1]
                out_stream = []
                for idx, a in enumerate(A):
                    if 3 <= idx < OT:
                        out_stream.append(pkt[idx])  # kt[idx] just before its pair
                    out_stream.append(a)
                    if pv and idx in (0, 1, 5, 7):
                        out_stream.append(pv.popleft())
                while pv:
                    out_stream.append(pv.popleft())
                out_stream.append(aflush)
                for p in out_stream:
                    p()

    nc.compile()
    return nc


def prep_host_inputs(inputs, bpc=BPC, cores=NCORES, group_sizes=GROUP_SIZES):
    """Shared (per-core-identical) tensors + per-core packed-x shards."""
    hs = np.ascontiguousarray(np.asarray(inputs["hidden_states"], dtype=np.float32))
    wq = np.asarray(inputs["wq"], np.float32)
    bq = np.asarray(inputs["bq"], np.float32)
    wk = np.asarray(inputs["wk"], np.float32)
    wv = np.asarray(inputs["wv"], np.float32)
    bias_table = np.asarray(inputs["bias_table"], np.float32)
    rel_index = np.asarray(inputs["rel_index"])

    bf = ml_dtypes.bfloat16
    scale = np.float32(1.0 / np.sqrt(HD))

    def pack_w(wT):  # [768, 768] -> [128, KT*768], k-tiles side by side
        return np.ascontiguousarray(
            np.concatenate([wT[k * 128 : (k + 1) * 128, :] for k in range(KT)], axis=1)
        )

    common = {
        "wqw": pack_w((wq.T * scale).astype(bf)),
        "wkw": pack_w(wk.T.astype(bf)),
        "wvw": pack_w(wv.T.astype(bf)),
        "bq2": np.ascontiguousarray((bq * scale).reshape(OT, 128).T),
    }
    rb = bias_table[rel_index]  # [i, j, H]
    ebT = np.exp(rb.transpose(2, 1, 0))  # [h, j, i]
    ebt = np.ones((H, 128, PW), np.float32)
    ebt[:, 0:128, 0:S] = ebT[:, 0:128, :]
    ebt[:, 0 : S - 128, IW : IW + S] = ebT[:, 128:S, :]
    common["ebias"] = np.ascontiguousarray(
        np.concatenate(list(ebt.astype(bf)), axis=1)
    )

    b0s = []
    acc = 0
    for GB in group_sizes:
        b0s.append(acc)
        acc += GB

    in_maps = []
    for c in range(cores):
        xc = hs[c * bpc : (c + 1) * bpc].reshape(bpc * S, D)
        xT = xc.T.astype(bf)  # [768, bpc*S]
        blocks = []
        for g, GB in enumerate(group_sizes):
            m0 = b0s[g] * S
            MG = GB * S
            for k in range(KT):
                blocks.append(xT[k * 128 : (k + 1) * 128, m0 : m0 + MG])
        in_maps.append(
            {"xP": np.ascontiguousarray(np.concatenate(blocks, axis=1)), **common}
        )
    return in_maps


_prog_cache = {}


def get_program(bpc=BPC, group_sizes=GROUP_SIZES):
    key = (bpc, group_sizes)
    if key not in _prog_cache:
        _prog_cache[key] = build_program(bpc, group_sizes)
    return _prog_cache[key]


def kernel(**inputs):
    nc = get_program()
    in_maps = prep_host_inputs(inputs)
    res = run_bass_kernel_spmd(nc, in_maps, list(range(NCORES)))
    out = np.concatenate(
        [res.results[c]["out"].astype(np.float32) for c in range(NCORES)], axis=0
    )
    bv = np.asarray(inputs["bv"], np.float32)
    return out.reshape(B, S, D) + bv
